# revision 26
# baseline (speedup 1.0000x reference)
"""Trainium2 Bass kernel: pre-norm transformer encoder block (B=2,N=2048,D=1024,
Hid=4096,H=16 heads, raw-reshape attention merge, shared LN params).

Sharding (8 cores, no collectives):
  core c: b = c//4, heads h = 4*(c%4)..4*(c%4)+3 of batch b.
  The raw o.reshape(B,N,D) merge maps head h exactly onto tokens
  [128h, 128h+128) of the residual stream, so each core's attention output
  lands on its own contiguous 512-token slice -> MLP is token-parallel with
  zero communication.

Performance notes (f32r baseline 555us -> 409us measured):
- all-bf16 matmul path: fast weight loads, half the DMA bytes/SBUF, and the
  PE clock stays warm (stuck-cold HAM stretches tracked the f32r phases).
- LayerNorm applied once by pre-scaling xn = (x-mu)*rstd (cheap bf16 DVE
  ops) instead of folding scale/shift into every matmul epilogue; rstd is
  exp(-0.5*ln(var+eps)) on ACT (Ln/Exp share a table with the softmax exp).
- attention inner loop software-pipelined by emission order: scores(i+2),
  exp(i+1), pv-accum(i), so the in-order PE queue never stalls behind the
  ACT-bound exp stream (ACT holds ~100% through both head pairs).
- softmax denominators: one batched DVE reciprocal per pair on a [128,32]
  layout via a DRAM roundtrip (DVE reciprocal cost scales with free size).
- DMA dispatch spread across sync/scalar/gpsimd queues; the exposed pair-1
  output scatter is split per head across DVE and gpsimd.
Scheduling here is fragile: per-engine queues execute in emission order, so
"obvious" rebalancing (k-parity engine splits, in-loop divisions) measured
SLOWER by creating cross-queue head-of-line stalls. Measure every change.
"""

from contextlib import ExitStack

import numpy as np
import ml_dtypes
import bass_rust
import concourse.bass as bass
import concourse.mybir as mybir
import concourse.tile as tile
from concourse.tile import TileContext, ScopedClock
from concourse.bass import ts

F32 = mybir.dt.float32
BF16 = mybir.dt.bfloat16
AF = mybir.ActivationFunctionType
OP = mybir.AluOpType

B, N, D, HID, H = 2, 2048, 1024, 4096, 16
DH = D // H            # 64
NCORES = 8
CPB = 4                # cores per batch
NH = 4                 # heads per core
TOK = N                # tokens per batch (attention span)
MY = 512               # tokens owned per core (MLP/residual)
P = 128
SL = 512               # free-dim slice for matmuls
NSL = TOK // SL        # 4
KD = D // P            # 8
NKT = TOK // P         # 16
HT = HID // P          # 32
EPS = 1e-5
EXP_SHIFT = -20.0      # constant logit shift; cancels in softmax, guards overflow

_PATCHED = False


def _patch_drain():
    """This walrus build rejects >2 sem waits on one instruction; split the
    Tile kernel-tail drain's waits across single-wait NOPs."""
    global _PATCHED
    if _PATCHED:
        return
    _PATCHED = True

    def _drain_and_barrier(self, tick_clock, wait_clock):
        gc = tick_clock.global_clock
        ticks = eval(repr(gc).replace("VectorClock", ""))
        n = len(ticks)
        for i, t in enumerate(ticks):
            if t > 0:
                single = [0] * n
                single[i] = t
                vc = bass_rust.VectorClock(single)
                nop = self.nc.sync.nop(nofuse=True, hint=f"drain_split_{i}")
                wait_clock.add_sem_waits(nop.ins, ScopedClock({None: vc}))
        self.nc.sync.drain()
        self.nc.all_engine_barrier()
        assert self.sems is not None
        popped = self.nc._tile_sem_poison_stack.pop()
        assert popped is self._sem_poison
        self.nc.clear_and_free_semaphores(list(self.sems.allocated().values()))
        self.nc.all_engine_barrier()

    TileContext._drain_and_barrier = _drain_and_barrier


def _split_excess_waits(nc):
    """This walrus build supports only one sync wait per instruction (two for
    EventSemaphore). Tile emits more; move the excess onto injected NoOps that
    run just before the instruction on the same engine."""
    nid = [0]
    for fn in nc.m.functions:
        for bb in fn.blocks:
            out = []
            changed = False
            for inst in bb.instructions:
                si = inst.sync_info
                waits = list(si.on_wait) if si is not None and si.on_wait else []
                cap = 2 if inst.opcode == "EventSemaphore" else 1
                if len(waits) > cap:
                    changed = True
                    for w in waits[:-cap]:
                        nid[0] += 1
                        nop = bass_rust.InstNoOp(
                            name=f"I-wsplit{nid[0]}", ins=[], outs=[])
                        nop.engine = inst.engine
                        nop.sync_info = bass_rust.SyncInfo(
                            on_wait=[w], on_update=[])
                        out.append(nop)
                    ups = list(si.on_update) if si.on_update else []
                    inst.sync_info = bass_rust.SyncInfo(
                        on_wait=waits[-cap:], on_update=ups)
                out.append(inst)
            if changed:
                bb.instructions = out


def build_program(split_waits=True):
    _patch_drain()
    nc = bass.Bass()

    xbf = nc.dram_tensor("xbf", [D, TOK], BF16, kind="ExternalInput")
    xTmy = nc.dram_tensor("xTmy", [D, MY], F32, kind="ExternalInput")
    wqk = nc.dram_tensor("wqk", [D, 4 * P], BF16, kind="ExternalInput")
    wv = nc.dram_tensor("wv", [D, NH * DH], BF16, kind="ExternalInput")
    bqk = nc.dram_tensor("bqk", [4 * P], F32, kind="ExternalInput")
    bvx = nc.dram_tensor("bvx", [NH * (DH + 1)], F32, kind="ExternalInput")
    w1 = nc.dram_tensor("w1", [D, HID], BF16, kind="ExternalInput")
    b1 = nc.dram_tensor("b1", [HID], F32, kind="ExternalInput")
    w2 = nc.dram_tensor("w2", [HID, D], BF16, kind="ExternalInput")
    b2 = nc.dram_tensor("b2", [D], F32, kind="ExternalInput")
    outT = nc.dram_tensor("outT", [D, MY], F32, kind="ExternalOutput")

    # DRAM scratch for row->partition broadcasts
    scr_rstd = nc.dram_tensor("scr_rstd", [TOK], BF16)
    scr_negmu = nc.dram_tensor("scr_negmu", [TOK], BF16)
    scr_rcp = nc.dram_tensor("scr_rcp", [16 * SL], F32)
    scr2_rstd = nc.dram_tensor("scr2_rstd", [MY], BF16)
    scr2_negmu = nc.dram_tensor("scr2_negmu", [MY], BF16)

    with TileContext(nc) as tc, ExitStack() as top:
        singles = top.enter_context(tc.tile_pool(name="singles", bufs=1))
        x2T_pool = top.enter_context(tc.tile_pool(name="x2T", bufs=1))

        ones_bf = singles.tile([P, 1], BF16)
        nc.vector.memset(ones_bf, 1.0)
        eps1 = singles.tile([1, 1], F32)
        nc.vector.memset(eps1, EPS)
        shiftP = singles.tile([P, 1], F32)
        nc.vector.memset(shiftP, EXP_SHIFT)
        b1_sb = singles.tile([P, HT], F32)
        nc.sync.dma_start(out=b1_sb, in_=b1.rearrange("(c p) -> p c", p=P))
        b2_sb = singles.tile([P, KD], F32)
        nc.sync.dma_start(out=b2_sb, in_=b2.rearrange("(c p) -> p c", p=P))
        bqk_sb = singles.tile([P, 4], F32)
        nc.sync.dma_start(out=bqk_sb, in_=bqk.rearrange("(c p) -> p c", p=P))
        bvxB = singles.tile([P, NH * (DH + 1)], F32)
        nc.gpsimd.dma_start(
            out=bvxB, in_=bvx[None, :].to_broadcast([P, NH * (DH + 1)]))

        # ================= Phase A: LN1 + qkv + v =================
        esAB = ExitStack()   # pools that live through phase B (qkvT, V')
        qkvT_pool = esAB.enter_context(tc.tile_pool(name="qkvT", bufs=1))
        vsb_pool = esAB.enter_context(tc.tile_pool(name="vsb", bufs=1))

        esA = ExitStack()    # phase-A only
        x_pool = esA.enter_context(tc.tile_pool(name="xp", bufs=1))
        xsq_pool = esA.enter_context(tc.tile_pool(name="xsq", bufs=2))
        xn_pool = esA.enter_context(tc.tile_pool(name="xn", bufs=1))
        w_pool = esA.enter_context(tc.tile_pool(name="wp", bufs=1))
        lnA = esA.enter_context(tc.tile_pool(name="lnA", bufs=1))
        bc_pool = esA.enter_context(tc.tile_pool(name="bcA", bufs=2))

        # resident weights (one DMA each)
        wqk_sb, wv_sb = [], []
        for k in range(KD):
            t = w_pool.tile([P, 4 * P], BF16, tag=f"wqk{k}")
            nc.gpsimd.dma_start(out=t, in_=wqk[ts(k, P), :])
            wqk_sb.append(t)
            t = w_pool.tile([P, NH * DH], BF16, tag=f"wv{k}")
            nc.gpsimd.dma_start(out=t, in_=wv[ts(k, P), :])
            wv_sb.append(t)

        # full x (bf16), loaded per (k, sl) slice
        x_sb = [[None] * KD for _ in range(NSL)]
        for sl in range(NSL):
            for k in range(KD):
                t = x_pool.tile([P, SL], BF16, name="xt", tag=f"x{sl}_{k}")
                nc.sync.dma_start(out=t, in_=xbf[ts(k, P), ts(sl, SL)])
                x_sb[sl][k] = t

        # qkvT col-tiles: 0=[q_h0;q_h1] 1=[q_h2;q_h3] 2=[k_h0;k_h1] 3=[k_h2;k_h3]
        qkvT = [qkvT_pool.tile([P, TOK], BF16, name=f"qkvT{ct}", tag=f"qkvT{ct}")
                for ct in range(4)]
        # vsb[nk]: [128 key-tokens, 4*(64+1)]; head h = cols [65h,65h+65),
        # col 65h+64 is the ones column (softmax denominator)
        vsb = [None] * NKT

        xn = [[None] * KD for _ in range(NSL)]

        def ln1_stats(sl, psA, row_pool):
            # squares (bf16 4x DVE)
            xsq = []
            for k in range(KD):
                t = xsq_pool.tile([P, SL], BF16, name="xsq", tag=f"xsq{k}")
                nc.vector.tensor_mul(t, x_sb[sl][k], x_sb[sl][k])
                xsq.append(t)
            s1p = psA.tile([1, SL], F32, tag="s1")
            s2p = psA.tile([1, SL], F32, tag="s2")
            for k in range(KD):
                nc.tensor.matmul(s1p, lhsT=ones_bf, rhs=x_sb[sl][k],
                                 start=(k == 0), stop=(k == KD - 1))
            for k in range(KD):
                nc.tensor.matmul(s2p, lhsT=ones_bf, rhs=xsq[k],
                                 start=(k == 0), stop=(k == KD - 1))
            negmu = row_pool.tile([1, SL], BF16, tag="negmu")
            nc.vector.tensor_scalar_mul(negmu, s1p, -1.0 / D)
            m2 = row_pool.tile([1, SL], F32, tag="m2")
            nc.vector.tensor_scalar_mul(m2, s2p, 1.0 / D)
            mu2 = row_pool.tile([1, SL], F32, tag="mu2")
            nc.vector.tensor_mul(mu2, negmu, negmu)
            var = row_pool.tile([1, SL], F32, tag="var")
            nc.vector.tensor_sub(var, m2, mu2)
            # rstd = exp(-0.5*ln(var+eps)); Ln and Exp share one ACT table
            lnv = row_pool.tile([1, SL], F32, tag="lnv")
            nc.scalar.activation(out=lnv, in_=var, func=AF.Ln,
                                 bias=eps1, scale=1.0)
            rstd = row_pool.tile([1, SL], BF16, tag="rstd")
            nc.scalar.activation(out=rstd, in_=lnv, func=AF.Exp,
                                 bias=0.0, scale=-0.5)
            nc.scalar.dma_start(out=scr_rstd[ts(sl, SL)], in_=rstd)
            nc.scalar.dma_start(out=scr_negmu[ts(sl, SL)], in_=negmu)
            rstdB = bc_pool.tile([P, SL], BF16, tag="rstdB")
            nc.gpsimd.dma_start(
                out=rstdB,
                in_=scr_rstd[ts(sl, SL)][None, :].to_broadcast([P, SL]))
            negmuB = bc_pool.tile([P, SL], BF16, tag="negmuB")
            nc.gpsimd.dma_start(
                out=negmuB,
                in_=scr_negmu[ts(sl, SL)][None, :].to_broadcast([P, SL]))
            # xn = (x - mu) * rstd   (bf16 4x DVE)
            for k in range(KD):
                xc = xsq_pool.tile([P, SL], BF16, name="xc", tag=f"xc{k}")
                nc.vector.tensor_add(xc, x_sb[sl][k], negmuB)
                t = xn_pool.tile([P, SL], BF16, name="xn", tag=f"xn{sl}_{k}")
                nc.vector.tensor_mul(t, xc, rstdB)
                xn[sl][k] = t

        def qkv_v(sl, psQ, psV):
            # kk columns first so attention can start earliest
            for ct in (2, 3, 0, 1):
                pq = psQ.tile([P, SL], F32, tag="pq")
                for k in range(KD):
                    nc.tensor.matmul(
                        pq, lhsT=wqk_sb[k][:, ts(ct, P)], rhs=xn[sl][k],
                        start=(k == 0), stop=(k == KD - 1))
                nc.scalar.activation(
                    out=qkvT[ct][:, ts(sl, SL)], in_=pq, func=AF.Identity,
                    bias=bqk_sb[:, ct:ct + 1], scale=1.0)
            for nkl in range(SL // P):
                nk = (SL // P) * sl + nkl
                pv = psV.tile([P, NH * DH], F32, tag="pv")
                for k in range(KD):
                    nc.tensor.matmul(
                        pv, lhsT=xn[sl][k][:, ts(nkl, P)], rhs=wv_sb[k],
                        start=(k == 0), stop=(k == KD - 1))
                vt = vsb_pool.tile([P, NH * (DH + 1)], BF16,
                                   name=f"v{nk}", tag=f"v{nk}")
                vr = vt.rearrange("p (h c) -> p h c", c=DH + 1)
                nc.vector.tensor_add(
                    vr[:, :, 0:DH],
                    pv.rearrange("p (h c) -> p h c", c=DH),
                    bvxB.rearrange("p (h c) -> p h c", c=DH + 1)[:, :, 0:DH])
                nc.vector.memset(vr[:, :, DH:DH + 1], 1.0)
                vsb[nk] = vt

        with (
            tc.tile_pool(name="psA", bufs=2, space="PSUM") as psA,
            tc.tile_pool(name="psQ", bufs=2, space="PSUM") as psQ,
            tc.tile_pool(name="psV", bufs=2, space="PSUM") as psV,
            tc.tile_pool(name="rowA", bufs=2) as rowA,
        ):
            # software-pipelined: stats(sl) ; qkv_v(sl-1)
            ln1_stats(0, psA, rowA)
            for sl in range(1, NSL):
                ln1_stats(sl, psA, rowA)
                qkv_v(sl - 1, psQ, psV)
            qkv_v(NSL - 1, psQ, psV)

        esA.close()   # free x, xsq, xn, weights, LN1 rows

        # ================= Phase B: attention =================
        x2T = [x2T_pool.tile([P, MY], F32, name=f"x2T{k}", tag=f"x2T{k}")
               for k in range(KD)]
        with ExitStack() as esB:
            psS = esB.enter_context(tc.tile_pool(name="psS", bufs=2, space="PSUM"))
            psO = esB.enter_context(tc.tile_pool(name="psO", bufs=2, space="PSUM"))
            pT_pool = esB.enter_context(tc.tile_pool(name="pT", bufs=6))
            oT_pool = esB.enter_context(tc.tile_pool(name="oT", bufs=1))
            rcp_pool = esB.enter_context(tc.tile_pool(name="rcp", bufs=2))
            xTmy_pool = esB.enter_context(tc.tile_pool(name="xTmyp", bufs=1))

            xTmy_sb = []
            for k in range(KD):
                t = xTmy_pool.tile([P, MY], F32, tag=f"xTmy{k}")
                nc.sync.dma_start(out=t, in_=xTmy[ts(k, P), :])
                xTmy_sb.append(t)

            NI = NSL * NKT   # 64 flattened (sl, nk) iterations per pair
            for pair in range(2):
                qq = qkvT[pair]
                kk = qkvT[2 + pair]
                h0 = 2 * pair
                oTs2 = [oT_pool.tile([P, TOK], F32, name=f"oTs{h}", tag=f"oT{h}")
                        for h in range(2)]
                den8 = oT_pool.tile([1, 2 * NSL * SL], F32, tag="den8")
                pou_t = {}
                ps2_l = [None] * NI
                pt2_l = [None] * NI
                po2_l = [None] * NSL

                def S(i):
                    sl, nk = divmod(i, NKT)
                    ps2 = psS.tile([P, 2 * SL], F32, name="ps2", tag="ps2")
                    nc.tensor.matmul(
                        ps2[:, 0:SL], lhsT=kk[0:64, ts(nk, P)],
                        rhs=qq[0:64, ts(sl, SL)],
                        start=True, stop=True, tile_position=(0, 0))
                    nc.tensor.matmul(
                        ps2[:, SL:2 * SL], lhsT=kk[64:128, ts(nk, P)],
                        rhs=qq[64:128, ts(sl, SL)],
                        start=True, stop=True, tile_position=(64, 0))
                    ps2_l[i] = ps2

                def E(i):
                    pt2 = pT_pool.tile([P, 2 * SL], BF16, name="pt2", tag="pt2")
                    nc.scalar.activation(out=pt2, in_=ps2_l[i], func=AF.Exp,
                                         bias=shiftP, scale=1.0)
                    pt2_l[i] = pt2

                def PP(i):
                    sl, nk = divmod(i, NKT)
                    if nk == 0:
                        po2_l[sl] = psO.tile([DH + 1, 2 * SL], F32,
                                             name="po2", tag="po2")
                    po2 = po2_l[sl]
                    pt2 = pt2_l[i]
                    nc.tensor.matmul(
                        po2[:, 0:SL],
                        lhsT=vsb[nk][:, ts(h0, DH + 1)], rhs=pt2[:, 0:SL],
                        start=(nk == 0), stop=(nk == NKT - 1))
                    nc.tensor.matmul(
                        po2[:, SL:2 * SL],
                        lhsT=vsb[nk][:, ts(h0 + 1, DH + 1)],
                        rhs=pt2[:, SL:2 * SL],
                        start=(nk == 0), stop=(nk == NKT - 1))
                    pt2_l[i] = None
                    ps2_l[i] = None

                def sl_tail(sl):
                    # drain PSUM promptly; stage denominators for one batched
                    # reciprocal at pair end
                    for h in range(2):
                        pou = rcp_pool.tile([DH + 1, SL], F32, name="pou",
                                            tag=f"pou{sl}_{h}")
                        nc.vector.tensor_copy(pou, po2_l[sl][:, ts(h, SL)])
                        pou_t[(sl, h)] = pou
                        nc.vector.tensor_copy(
                            den8[0:1, ts(2 * sl + h, SL)], pou[DH:DH + 1, :])

                # software-pipelined emission: scores run one iteration ahead
                # of exp; po lags one behind, so the in-order PE queue never
                # stalls on the ACT engine
                S(0)
                E(0)
                S(1)
                for i in range(NI):
                    if i + 2 < NI:
                        S(i + 2)
                    if i + 1 < NI:
                        E(i + 1)
                    PP(i)
                    if i % NKT == NKT - 1:
                        sl_tail(i // NKT)

                # pair epilogue: one batched reciprocal for all 8 denominators,
                # spread over 128 partitions via a DRAM roundtrip (DVE
                # reciprocal cost scales with free size only)
                nc.scalar.dma_start(out=scr_rcp[0:2 * NSL * SL], in_=den8)
                denp = oT_pool.tile([P, 2 * NSL * SL // P], F32, tag="denp")
                nc.scalar.dma_start(
                    out=denp,
                    in_=scr_rcp[0:2 * NSL * SL]
                    .rearrange("(p c) -> p c", p=P))
                rcpp = oT_pool.tile([P, 2 * NSL * SL // P], F32, tag="rcpp")
                nc.vector.reciprocal(rcpp, denp)
                nc.scalar.dma_start(
                    out=scr_rcp[0:2 * NSL * SL]
                    .rearrange("(p c) -> p c", p=P),
                    in_=rcpp)
                for sl in range(NSL):
                    for h in range(2):
                        slot = 2 * sl + h
                        rcpB = rcp_pool.tile([DH, SL], F32, tag=f"rcpB{h}")
                        nc.sync.dma_start(
                            out=rcpB,
                            in_=scr_rcp[ts(slot, SL)][None, :]
                            .to_broadcast([DH, SL]))
                        oTs = oTs2[h]
                        nc.vector.tensor_mul(oTs[0:64, ts(sl, SL)],
                                             pou_t[(sl, h)][0:DH, :], rcpB)
                        nc.gpsimd.dma_start(out=oTs[64:128, ts(sl, SL)],
                                            in_=oTs[0:64, ts(sl, SL)])
                # scatter both heads' outputs into x2T via strided views:
                # attn_out^T[64j+d, m] = oT[d, 16m+j].  pair 0 scatters on
                # DVE (hidden under pair-1 attention); pair 1 on gpsimd so
                # the B->C boundary isn't DVE-serialized
                for h in range(2):
                    # pair-0 scatter hides under pair-1's attention on DVE;
                    # pair-1's is exposed at the B->C boundary, so its two
                    # heads run on different engines (contiguous 16-op blocks)
                    eng = nc.vector if (pair == 0 or h == 1) else nc.gpsimd
                    hh = 2 * pair + h
                    c0 = P * hh
                    ov = oTs2[h].rearrange("p (m j) -> p m j", j=16)
                    for k in range(KD):
                        eng.tensor_add(
                            x2T[k][0:64, c0:c0 + P],
                            xTmy_sb[k][0:64, c0:c0 + P],
                            ov[0:64, :, 2 * k])
                        eng.tensor_add(
                            x2T[k][64:128, c0:c0 + P],
                            xTmy_sb[k][64:128, c0:c0 + P],
                            ov[64:128, :, 2 * k + 1])
        esAB.close()  # free qkvT, V'

        # ================= Phase C: LN2 + MLP =================
        ln2 = top.enter_context(tc.tile_pool(name="ln2", bufs=1))
        x2b_pool = top.enter_context(tc.tile_pool(name="x2b", bufs=1))
        with (
            tc.tile_pool(name="psL", bufs=1, space="PSUM") as psL,
            tc.tile_pool(name="sq2", bufs=1) as sq2_pool,
            tc.tile_pool(name="row2", bufs=1) as row2_pool,
        ):
            # bf16 copy of x2 for fast stats + matmul streaming (on ACT:
            # DVE is busy with the pair-1 epilogue at this point)
            x2h = []
            for k in range(KD):
                t = sq2_pool.tile([P, MY], BF16, name="x2h", tag=f"x2h{k}")
                if k % 2 == 0:
                    nc.scalar.copy(t, x2T[k])
                else:
                    nc.vector.tensor_copy(t, x2T[k])
                x2h.append(t)
            s1p = psL.tile([1, MY], F32, tag="s1")
            s2p = psL.tile([1, MY], F32, tag="s2")
            for k in range(KD):
                nc.tensor.matmul(s1p, lhsT=ones_bf, rhs=x2h[k],
                                 start=(k == 0), stop=(k == KD - 1))
            xsq2 = []
            for k in range(KD):
                t = sq2_pool.tile([P, MY], BF16, name="xsq2", tag=f"xsq2{k}")
                nc.vector.tensor_mul(t, x2h[k], x2h[k])
                xsq2.append(t)
            for k in range(KD):
                nc.tensor.matmul(s2p, lhsT=ones_bf, rhs=xsq2[k],
                                 start=(k == 0), stop=(k == KD - 1))
            negmu2 = row2_pool.tile([1, MY], BF16, tag="negmu2")
            nc.vector.tensor_scalar_mul(negmu2, s1p, -1.0 / D)
            m2 = row2_pool.tile([1, MY], F32, tag="m2b")
            nc.vector.tensor_scalar_mul(m2, s2p, 1.0 / D)
            mu22 = row2_pool.tile([1, MY], F32, tag="mu22")
            nc.vector.tensor_mul(mu22, negmu2, negmu2)
            var = row2_pool.tile([1, MY], F32, tag="var2")
            nc.vector.tensor_sub(var, m2, mu22)
            lnv2 = row2_pool.tile([1, MY], F32, tag="lnv2")
            nc.scalar.activation(out=lnv2, in_=var, func=AF.Ln,
                                 bias=eps1, scale=1.0)
            rstd2 = row2_pool.tile([1, MY], BF16, tag="rstd2")
            nc.scalar.activation(out=rstd2, in_=lnv2, func=AF.Exp,
                                 bias=0.0, scale=-0.5)
            nc.scalar.dma_start(out=scr2_rstd[:], in_=rstd2)
            nc.scalar.dma_start(out=scr2_negmu[:], in_=negmu2)
            rstd2B = ln2.tile([P, MY], BF16)
            nc.gpsimd.dma_start(
                out=rstd2B, in_=scr2_rstd[None, :].to_broadcast([P, MY]))
            negmu2B = ln2.tile([P, MY], BF16)
            nc.gpsimd.dma_start(
                out=negmu2B, in_=scr2_negmu[None, :].to_broadcast([P, MY]))

            # xn2 = (x2 - mu)*rstd in bf16 for the MLP
            x2b = []
            for k in range(KD):
                xc = sq2_pool.tile([P, MY], BF16, name="xc2", tag=f"xsq2{k}")
                nc.vector.tensor_add(xc, x2h[k], negmu2B)
                t = x2b_pool.tile([P, MY], BF16, name=f"x2b{k}", tag=f"x2b{k}")
                nc.vector.tensor_mul(t, xc, rstd2B)
                x2b.append(t)

        with (
            tc.tile_pool(name="psF", bufs=2, space="PSUM") as psF,
            tc.tile_pool(name="w1sb", bufs=1) as w1_pool,
            tc.tile_pool(name="hT", bufs=1) as hT_pool,
            tc.tile_pool(name="fctmp", bufs=2) as fctmp_pool,
            tc.tile_pool(name="w2sb", bufs=2) as w2_pool,
        ):
            GK = 8           # hid col groups of 512
            GW = HID // GK   # 512

            def w1_dma(gk):
                w1sb = []
                for k in range(KD):
                    t = w1_pool.tile([P, GW], BF16, name="w1t",
                                     tag=f"w1_{k}_{gk % 2}")
                    nc.gpsimd.dma_start(out=t, in_=w1[ts(k, P), ts(gk, GW)])
                    w1sb.append(t)
                return w1sb

            w2r = w2.rearrange("(c p) d -> p c d", p=P)   # [128, 32, 1024]

            def w2_dma(c):
                kd, half = divmod(c, 2)
                t = w2_pool.tile([P, HT // 2, P], BF16, name="w2t", tag="w2sb")
                nc.scalar.dma_start(
                    out=t, in_=w2r[:, ts(half, HT // 2), ts(kd, P)])
                return t

            # prefetch first fc1 weight group and first fc2 chunks while the
            # LN2 chain runs
            w1sb_next = w1_dma(0)
            w2_t = [w2_dma(0), w2_dma(1)]

            hT = [None] * HT
            for gk in range(GK):
                w1sb = w1sb_next
                if gk + 1 < GK:
                    w1sb_next = w1_dma(gk + 1)
                for khl in range(GW // P):
                    kh = (GW // P) * gk + khl
                    pf = psF.tile([P, MY], F32, tag="pf")
                    for k in range(KD):
                        nc.tensor.matmul(
                            pf, lhsT=w1sb[k][:, ts(khl, P)], rhs=x2b[k],
                            start=(k == 0), stop=(k == KD - 1))
                    ht = hT_pool.tile([P, MY], BF16, name="ht", tag=f"hT{kh}")
                    nc.scalar.activation(out=ht, in_=pf, func=AF.Gelu,
                                         bias=b1_sb[:, kh:kh + 1], scale=1.0)
                    hT[kh] = ht

            for kd in range(KD):
                pf = psF.tile([P, MY], F32, tag="pf2")
                for half in range(2):
                    c = 2 * kd + half
                    w2h = w2_t[c % 2]
                    for khl in range(HT // 2):
                        kh = half * (HT // 2) + khl
                        nc.tensor.matmul(
                            pf, lhsT=w2h[:, khl, :], rhs=hT[kh],
                            start=(kh == 0), stop=(kh == HT - 1))
                    if c + 2 < 2 * KD:
                        w2_t[c % 2] = w2_dma(c + 2)
                t = fctmp_pool.tile([P, MY], F32, tag="fco")
                nc.scalar.activation(out=t, in_=pf, func=AF.Identity,
                                     bias=b2_sb[:, kd:kd + 1], scale=1.0)
                ot = fctmp_pool.tile([P, MY], F32, tag="fcout")
                nc.vector.tensor_add(ot, t, x2T[kd])
                nc.sync.dma_start(out=outT[ts(kd, P), :], in_=ot)

    if split_waits:
        _split_excess_waits(nc)
    return nc


def host_prep(x, w_qkv, b_qkv, ln_g, ln_b, w1, b1, w2, b2):
    """Fold LN affine params into weights; build per-core input maps."""
    x = np.asarray(x, np.float32)
    w_qkv = np.asarray(w_qkv, np.float32)
    b_qkv = np.asarray(b_qkv, np.float32)
    ln_g = np.asarray(ln_g, np.float32)
    ln_b = np.asarray(ln_b, np.float32)
    w1 = np.asarray(w1, np.float32)
    b1 = np.asarray(b1, np.float32)
    w2 = np.asarray(w2, np.float32)
    b2 = np.asarray(b2, np.float32)

    wqkv_eff = ln_g[:, None] * w_qkv
    bqkv_eff = b_qkv + ln_b @ w_qkv
    w1_eff = np.ascontiguousarray(ln_g[:, None] * w1)
    b1_eff = b1 + ln_b @ w1

    in_maps = []
    for c in range(NCORES):
        b = c // CPB
        heads = [4 * (c % CPB) + i for i in range(NH)]
        qcols = np.concatenate([np.arange(h * DH, (h + 1) * DH) for h in heads])
        kcols = qcols + D
        vcols = qcols + 2 * D
        qkcols = np.concatenate([qcols, kcols])
        xb = x[b]
        my0 = MY * (c % CPB)
        bvx = np.zeros(NH * (DH + 1), np.float32)
        bv = bqkv_eff[vcols]
        for h in range(NH):
            bvx[h * (DH + 1):h * (DH + 1) + DH] = bv[h * DH:(h + 1) * DH]
        in_maps.append({
            "xbf": np.ascontiguousarray(xb.T).astype(ml_dtypes.bfloat16),
            "xTmy": np.ascontiguousarray(xb[my0:my0 + MY].T),
            "wqk": np.ascontiguousarray(
                wqkv_eff[:, qkcols]).astype(ml_dtypes.bfloat16),
            "wv": np.ascontiguousarray(
                wqkv_eff[:, vcols]).astype(ml_dtypes.bfloat16),
            "bqk": np.ascontiguousarray(bqkv_eff[qkcols]),
            "bvx": bvx,
            "w1": w1_eff.astype(ml_dtypes.bfloat16),
            "b1": b1_eff,
            "w2": w2.astype(ml_dtypes.bfloat16),
            "b2": b2,
        })
    return in_maps


_NC_CACHE = None


def kernel(x, w_qkv, b_qkv, ln_g, ln_b, w1, b1, w2, b2):
    global _NC_CACHE
    from concourse.bass_utils import run_bass_kernel_spmd

    if _NC_CACHE is None:
        _NC_CACHE = build_program()
    nc = _NC_CACHE
    in_maps = host_prep(x, w_qkv, b_qkv, ln_g, ln_b, w1, b1, w2, b2)
    res = run_bass_kernel_spmd(nc, in_maps, list(range(NCORES))).results

    out = np.empty((B, N, D), np.float32)
    for c in range(NCORES):
        b = c // CPB
        my0 = MY * (c % CPB)
        out[b, my0:my0 + MY, :] = res[c]["outT"].T
    return out
